# revision 25
# baseline (speedup 1.0000x reference)
"""Bass/Trainium2 kernel for nn_EnergyModel (3-layer GAT + MLP head).

Sharding: data-parallel over batch B=32 across 8 NeuronCores (4 graphs/core),
GAT/MLP params replicated.

Key design (v3):
  - Host pre-transposes bonds to [g, j', b=(2r+jh), i] so the attention mask
    loads directly in the transposed layout the PE aggregation needs — no
    on-device transposes. Mask is MULTIPLICATIVE: Et = exp(prelu(S)) * bond.
  - Host folds W@a into Wsd [cin, 10] so src/dst logits come straight from
    atomsT in one matmul: sd = Wsd.T @ atomsT (rows 0-4 src_r, 5-9 dst_r).
  - S[j', i] per block (r, jh) via rank-2 augmented outer product
    [dst|ones]^T [ones|src] (f32r).
  - prelu (ACT) -> exp (ACT, bf16 out) -> mask-mult (DVE, bf16 2x mode).
  - aggregation out^T = sum_b h_b^T Et_b and Z = ones^T Et on PE in bf16.
  - Graphs processed in interleaved PAIRS (per-layer alternation) so every
    engine has the sibling graph's work queued -> fills dependency bubbles,
    keeps PE warm (HAM).
  - sd gathers DMA-triggered from the Vector engine right after its own sd
    eviction (avoids sync-queue convoys); mask int->bf16 cast on DVE
    (2x_2p single-src mode); aug-tile ones rows memset only once per
    physical buffer; fast reciprocal.
"""

import sys
from contextlib import ExitStack

if "/opt/trn_rl_repo" not in sys.path:
    sys.path.insert(0, "/opt/trn_rl_repo")

import numpy as np

B, N, CIN, C, R, XD = 32, 256, 64, 128, 5, 1024
NCORE = 8
NG = B // NCORE  # graphs per core
NRC = R * C      # 640
NB = 2 * R       # 10 blocks b = 2r + jh
H1 = 256         # MLP hidden 1
H2 = 32          # MLP hidden 2
ZDIM = 2 * C + XD  # 1280

_BUILD_CACHE = {}


def build(n_graphs=NG, with_bias=True, repeat=1):
    key = (n_graphs, with_bias, repeat)
    if key in _BUILD_CACHE:
        return _BUILD_CACHE[key]

    import concourse.bass as bass
    from concourse import bacc
    import concourse.tile as tile
    import concourse.mybir as mybir
    from concourse.masks import make_identity

    f32 = mybir.dt.float32
    f32r = mybir.dt.float32r
    bf16 = mybir.dt.bfloat16
    i32 = mybir.dt.int32
    AF = mybir.ActivationFunctionType
    OP = mybir.AluOpType

    def mm(out, lhsT, rhs, **kw):
        nc.tensor.matmul(out, lhsT, rhs, **kw)

    nc = bacc.Bacc("TRN2", target_bir_lowering=False)
    ng = n_graphs

    atoms_d = nc.dram_tensor("y_atoms", [ng, N, CIN], f32, kind="ExternalInput")
    bondsT_d = nc.dram_tensor("bonds_t", [ng, 128, NB, N], i32, kind="ExternalInput")
    x_d = nc.dram_tensor("x", [ng, XD], f32, kind="ExternalInput")
    W_d = [
        nc.dram_tensor("W1", [CIN, NRC], f32, kind="ExternalInput"),
        nc.dram_tensor("W2", [C, NRC], f32, kind="ExternalInput"),
        nc.dram_tensor("W3", [C, NRC], f32, kind="ExternalInput"),
    ]
    Wsd_d = [
        nc.dram_tensor(f"Wsd{i}", [CIN if i == 1 else C, NB], f32,
                       kind="ExternalInput")
        for i in (1, 2, 3)
    ]
    We1_d = nc.dram_tensor("We1", [ZDIM, H1], f32, kind="ExternalInput")
    We2_d = nc.dram_tensor("We2", [H1, H2], f32, kind="ExternalInput")
    We3_d = nc.dram_tensor("We3", [H2, 1], f32, kind="ExternalInput")
    if with_bias:
        b_d = [
            nc.dram_tensor(f"b{i}", [1, NRC], f32, kind="ExternalInput")
            for i in (1, 2, 3)
        ]
        bsd_d = [
            nc.dram_tensor(f"bsd{i}", [1, NB], f32, kind="ExternalInput")
            for i in (1, 2, 3)
        ]
        be1_d = nc.dram_tensor("be1", [1, H1], f32, kind="ExternalInput")
        be2_d = nc.dram_tensor("be2", [1, H2], f32, kind="ExternalInput")
        be3_d = nc.dram_tensor("be3", [1, 1], f32, kind="ExternalInput")
    out_d = nc.dram_tensor("out", [ng, 1], f32, kind="ExternalOutput")

    with tile.TileContext(nc) as tc, ExitStack() as ctx:
        const = ctx.enter_context(tc.tile_pool(name="const", bufs=1))
        gpool = ctx.enter_context(tc.tile_pool(name="gpool", bufs=2))
        mpool = ctx.enter_context(tc.tile_pool(name="mpool", bufs=4))
        gpool3 = ctx.enter_context(tc.tile_pool(name="gpool3", bufs=3))
        spool = ctx.enter_context(tc.tile_pool(name="spool", bufs=2))
        ps_s = ctx.enter_context(tc.tile_pool(name="ps_s", bufs=2, space="PSUM"))
        ps_sm = ctx.enter_context(tc.tile_pool(name="ps_sm", bufs=4, space="PSUM"))

        # ---------------- prologue: start pair-0 loads first ----------
        preloaded = {}
        for g in (0, 1):
            bT_pre = gpool.tile([128, NB, N], i32, tag="bondsT")
            nc.sync.dma_start(bT_pre[:], bondsT_d[g])
            preloaded[g] = bT_pre
        pre_atoms = {}
        for g in (0, 1):
            at_pre = spool.tile([128, 2, CIN], f32, tag="atnat")
            for ib in range(2):
                nc.sync.dma_start(at_pre[:, ib, :],
                                  atoms_d[g, ib * 128:(ib + 1) * 128, :])
            pre_atoms[g] = at_pre

        # ---------------- constants (layer-critical first) ----------------
        ident = const.tile([128, 128], f32)
        make_identity(nc, ident[:])
        onesf = const.tile([128, 1], f32)
        nc.vector.memset(onesf[:], 1.0)
        ones_bf = const.tile([128, 1], bf16)
        nc.vector.memset(ones_bf[:], 1.0)
        onesrf = const.tile([1, 256], f32)
        nc.vector.memset(onesrf[:], 1.0)
        ones_row = const.tile([1, 256], f32r)
        nc.vector.tensor_copy(ones_row[:], onesrf[:])

        W_sb = []
        Wsd_sb = []
        for li in range(3):
            cin = CIN if li == 0 else C
            wsd_raw = spool.tile([cin, NB], f32, tag="wsd_raw")
            nc.sync.dma_start(wsd_raw[:], Wsd_d[li][:])
            wsd = const.tile([cin, NB], bf16, tag=f"Wsd{li}")
            nc.vector.tensor_copy(wsd[:], wsd_raw[:])
            Wsd_sb.append(wsd)
        for li in range(3):
            cin = CIN if li == 0 else C
            w_raw = spool.tile([cin, NRC], f32, tag="w_raw")
            nc.sync.dma_start(w_raw[:], W_d[li][:])
            w = const.tile([cin, NRC], bf16, tag=f"W{li}")
            nc.vector.tensor_copy(w[:], w_raw[:])
            W_sb.append(w)

        # MLP weights (needed only at the very end) — loaded late via the
        # deferred flag below
        We1_raw = const.tile([128, 10, H1], f32)
        We1_sb = const.tile([128, 10, H1], f32r)
        We2_sb = const.tile([128, 2, H2], f32)
        We3_sb = const.tile([H2, 1], f32)

        def load_mlp_weights():
            nc.sync.dma_start(We1_raw[:],
                              We1_d.rearrange("(kb p) n -> p kb n", p=128))
            nc.vector.tensor_copy(We1_sb[:], We1_raw[:])
            nc.sync.dma_start(We2_sb[:],
                              We2_d.rearrange("(kb p) n -> p kb n", p=128))
            nc.sync.dma_start(We3_sb[:], We3_d[:])

        if with_bias:
            b_row = []
            bsd_row = []
            for li in range(3):
                braw = spool.tile([1, NRC], f32, tag="braw")
                nc.sync.dma_start(braw[:], b_d[li][:])
                br = const.tile([1, NRC], f32r, tag=f"brow{li}")
                nc.vector.tensor_copy(br[:], braw[:])
                b_row.append(br)
                bsraw = spool.tile([1, NB], f32, tag="bsraw")
                nc.sync.dma_start(bsraw[:], bsd_d[li][:])
                bsr = const.tile([1, NB], f32r, tag=f"bsdrow{li}")
                nc.vector.tensor_copy(bsr[:], bsraw[:])
                bsd_row.append(bsr)
            beraw = spool.tile([1, H1], f32, tag="beraw")
            nc.sync.dma_start(beraw[:], be1_d[:])
            be1_row = const.tile([1, H1], f32r)
            nc.vector.tensor_copy(be1_row[:], beraw[:])
            be2_row = const.tile([1, H2], f32)
            nc.sync.dma_start(be2_row[:], be2_d[:])
            be3_row = const.tile([1, 1], f32)
            nc.sync.dma_start(be3_row[:], be3_d[:])

        # MLP lhsT staging: z^T chunks [128, kb, g]; kb 0..7 = x, 8 = mean, 9 = max
        zT = const.tile([128, 10, ng], f32r)

        # ---------------- per-graph helpers ----------------
        def setup_graph(g, first_pair):
            st = {}
            if g in preloaded:
                bT_sb = preloaded[g]
            else:
                bT_sb = gpool.tile([128, NB, N], i32, tag="bondsT")
                nc.sync.dma_start(bT_sb[:], bondsT_d[g])
            Mk = mpool.tile([128, NB, N], bf16, tag="mask")
            nc.vector.tensor_copy(
                Mk[:].rearrange("p a b -> p (a b)"),
                bT_sb[:].rearrange("p a b -> p (a b)"),
            )
            st["Mk"] = Mk

            dstP_g = []
            srcP_g = []
            for db in range(2):
                dP = gpool.tile([2, R, 256], bf16, tag=f"dstp{db}")
                sP = gpool.tile([2, R, 256], bf16, tag=f"srcp{db}")
                if first_pair:
                    # ones rows; physical buffers are reused intact by the
                    # second pair (gathers only overwrite the data rows)
                    nc.gpsimd.memset(dP[:], 1.0)
                    nc.gpsimd.memset(sP[:], 1.0)
                dstP_g.append(dP)
                srcP_g.append(sP)
            st["dstP"] = dstP_g
            st["srcP"] = srcP_g

            if g in pre_atoms:
                at_nat = pre_atoms[g]
            else:
                at_nat = spool.tile([128, 2, CIN], f32, tag="atnat")
                for ib in range(2):
                    nc.sync.dma_start(at_nat[:, ib, :],
                                      atoms_d[g, ib * 128:(ib + 1) * 128, :])
            atT_ps = ps_sm.tile([CIN, 2, 128], f32, tag="sm")
            for ib in range(2):
                nc.tensor.matmul(
                    atT_ps[:, ib, :], at_nat[:, ib, :], ident[:],
                    is_transpose=True, start=True, stop=True,
                )
            atoms_cur = gpool.tile([CIN, 256], bf16, tag="atoms0")
            nc.vector.tensor_copy(
                atoms_cur[:], atT_ps.rearrange("c a b -> c (a b)")
            )
            st["atoms"] = atoms_cur

            x_stage = spool.tile([128, 8], f32, tag="xstage")
            nc.sync.dma_start(x_stage[:], x_d[g].rearrange("(f p) -> p f", p=128))
            nc.vector.tensor_copy(
                zT[:, 0:8, g:g + 1].rearrange("p a b -> p (a b)"), x_stage[:])
            return st

        def gat_layer_pair(sts, li):
            """One GAT layer for a PAIR of graphs, step-interleaved so both
            graphs' work is adjacent in every engine queue."""
            W = W_sb[li]
            # -- sd first (its DVE eviction gates the S matmuls via gathers) --
            for st in sts:
                sd_ps = ps_sm.tile([NB, 256], f32, tag="sm")
                mm(sd_ps[:], Wsd_sb[li][:], st["atoms"],
                   start=True, stop=not with_bias)
                if with_bias:
                    mm(sd_ps[:], bsd_row[li][:], ones_row[:],
                       start=False, stop=True)
                st["sd_ps"] = sd_ps
            for st in sts:
                sd_sb = spool.tile([NB, 256], bf16, tag="sdsb")
                nc.vector.tensor_copy(sd_sb[:], st["sd_ps"][:])
                dstP = st["dstP"][li % 2]
                srcP = st["srcP"][li % 2]
                nc.gpsimd.dma_start(dstP[0:1], sd_sb[R:NB, :])
                nc.gpsimd.dma_start(srcP[1:2], sd_sb[0:R, :])

            # -- h = atoms @ W: out [i, (r,c)]; evicted to bf16 --
            for st in sts:
                h_bf = gpool3.tile([128, 2, NRC], bf16, tag="h")
                for ib in range(2):
                    hA = ps_sm.tile([128, 384], f32, tag="sm")
                    hB = ps_sm.tile([128, 256], f32, tag="sm")
                    lt = st["atoms"][:, ib * 128:(ib + 1) * 128]
                    mm(hA[:], lt, W[:, 0:384], start=True, stop=not with_bias)
                    mm(hB[:], lt, W[:, 384:NRC], start=True, stop=not with_bias)
                    if with_bias:
                        mm(hA[:], ones_row[:, :128], b_row[li][:, 0:384],
                           start=False, stop=True)
                        mm(hB[:], ones_row[:, :128], b_row[li][:, 384:NRC],
                           start=False, stop=True)
                    if ib == 0:
                        nc.vector.tensor_copy(h_bf[:, ib, 0:384], hA[:])
                        nc.scalar.activation(h_bf[:, ib, 384:NRC], hB[:], AF.Copy)
                    else:
                        nc.scalar.activation(h_bf[:, ib, 0:384], hA[:], AF.Copy)
                        nc.vector.tensor_copy(h_bf[:, ib, 384:NRC], hB[:])
                st["h_bf"] = h_bf

            # -- S -> prelu -> exp -> mask, chunk- and graph-interleaved;
            # agg/Z accumulation MMs software-pipelined one chunk behind so
            # the PE stream stays dense (keeps HAM warm) --
            CHUNKS = (4, 4, 2)
            STARTS = (0, 4, 8)
            for st in sts:
                Et = gpool3.tile([128, NB, 256], bf16, tag="et")
                st["Et"] = Et
                o_ps = ps_sm.tile([128, 256], f32, tag="sm")
                z_ps = ps_sm.tile([1, 256], f32, tag="sm")
                st["o_ps"] = o_ps
                st["z_ps"] = z_ps

            def agg_chunk(ci):
                nb, b0 = CHUNKS[ci], STARTS[ci]
                for st in sts:
                    h_bf = st["h_bf"]
                    Et = st["Et"]
                    for k in range(nb):
                        b = b0 + k
                        r, jh = b // 2, b % 2
                        mm(st["o_ps"][:], h_bf[:, jh, r * 128:(r + 1) * 128],
                           Et[:, b, :], start=(b == 0), stop=(b == NB - 1),
                           skip_group_check=True)
                    for k in range(nb):
                        b = b0 + k
                        mm(st["z_ps"][:], ones_bf[:],
                           Et[:, b, :], start=(b == 0), stop=(b == NB - 1),
                           skip_group_check=True)

            for ci, nb in enumerate(CHUNKS):
                b0 = STARTS[ci]
                for st in sts:
                    dstP = st["dstP"][li % 2]
                    srcP = st["srcP"][li % 2]
                    S_ps = ps_s.tile([128, 4, 256], f32, tag="sps")
                    for k in range(nb):
                        b = b0 + k
                        r, jh = b // 2, b % 2
                        mm(S_ps[:, k, :],
                           dstP[0:2, r, jh * 128:(jh + 1) * 128],
                           srcP[0:2, r, :],
                           start=True, stop=True)
                    st["S_ps"] = S_ps
                if ci > 0:
                    agg_chunk(ci - 1)
                for st in sts:
                    L_sb = gpool3.tile([128, 4, 256], f32, tag="lsb")
                    nc.scalar.activation(
                        L_sb[:, 0:nb].rearrange("p a b -> p (a b)"),
                        st["S_ps"][:, 0:nb].rearrange("p a b -> p (a b)"),
                        AF.Prelu, alpha=0.2,
                    )
                    st["L_sb"] = L_sb
                for st in sts:
                    Eu = gpool3.tile([128, 4, 256], bf16, tag="eu")
                    nc.scalar.activation(
                        Eu[:, 0:nb].rearrange("p a b -> p (a b)"),
                        st["L_sb"][:, 0:nb].rearrange("p a b -> p (a b)"),
                        AF.Exp,
                    )
                    st["Eu"] = Eu
                for st in sts:
                    nc.vector.tensor_tensor(
                        st["Et"][:, b0:b0 + nb].rearrange("p a b -> p (a b)"),
                        st["Eu"][:, 0:nb].rearrange("p a b -> p (a b)"),
                        st["Mk"][:, b0:b0 + nb].rearrange("p a b -> p (a b)"),
                        op=OP.mult,
                    )
            agg_chunk(len(CHUNKS) - 1)

            # -- normalize (+ inter-layer leaky) --
            for st in sts:
                rz_f = spool.tile([1, 256], f32, tag="rzf")
                nc.vector.reciprocal_approx_fast(rz_f[:], st["z_ps"][:])
                rz_sb = spool.tile([1, 256], f32r, tag="rz")
                nc.vector.tensor_copy(rz_sb[:], rz_f[:])
                st["rz"] = rz_sb
            for st in sts:
                O_sb = spool.tile([128, 256], f32, tag="osb")
                if li < 2:
                    nc.scalar.activation(O_sb[:], st["o_ps"][:], AF.Prelu,
                                         alpha=0.2)
                else:
                    nc.scalar.activation(O_sb[:], st["o_ps"][:], AF.Copy)
                st["O_sb"] = O_sb
            for st in sts:
                rzb_ps = ps_sm.tile([128, 256], f32, tag="sm")
                mm(rzb_ps[:], ones_row[:, :128], st["rz"][:],
                   start=True, stop=True)
                nxt = gpool.tile([C, 256], bf16, tag=f"atoms{li + 1}")
                nc.vector.tensor_tensor(nxt[:], st["O_sb"][:], rzb_ps[:],
                                        op=OP.mult)
                st["atoms"] = nxt

        def feats(st, g):
            h3T = st["atoms"]
            mean_raw = spool.tile([128, 1], f32, tag="mean")
            nc.vector.tensor_reduce(mean_raw[:], h3T[:],
                                    axis=mybir.AxisListType.X, op=OP.add)
            nc.vector.tensor_scalar(zT[:, 8, g:g + 1], mean_raw[:], 1.0 / N,
                                    None, op0=OP.mult)
            nc.vector.tensor_reduce(zT[:, 9, g:g + 1], h3T[:],
                                    axis=mybir.AxisListType.X, op=OP.max)

        # ---------------- interleaved graph pairs ----------------
        for _rep in range(repeat):
         for gp in range(ng // 2):
            pair = (2 * gp, 2 * gp + 1)
            if gp == 0:
                sts = [setup_graph(g, True) for g in pair]
            else:
                sts = next_sts  # prefetched during previous pair's layer 0
            for li in range(3):
                gat_layer_pair(sts, li)
                if li == 0 and gp + 1 < ng // 2:
                    next_sts = [setup_graph(g, False)
                                for g in (2 * gp + 2, 2 * gp + 3)]
                if li == 1 and gp == 0:
                    load_mlp_weights()
            for st, g in zip(sts, pair):
                feats(st, g)

         # ---------------- MLP head (batched over graphs) ---------------
         zz_ps = ps_sm.tile([ng, H1], f32, tag="sm")
         for kb in range(10):
            mm(zz_ps[:], zT[:, kb, :], We1_sb[:, kb, :],
               start=(kb == 0), stop=(kb == 9) and not with_bias)
         if with_bias:
            mm(zz_ps[:], ones_row[:, :ng], be1_row[:], start=False, stop=True)
         zzl = spool.tile([ng, H1], f32, tag="zzl")
         nc.scalar.activation(zzl[:], zz_ps[:], AF.Prelu, alpha=0.2)
         zzT_ps = ps_sm.tile([128, 2, ng], f32, tag="sm")
         for hh in range(2):
            nc.tensor.matmul(zzT_ps[:, hh, :], zzl[:, hh * 128:(hh + 1) * 128],
                             ident[:ng, :ng], is_transpose=True,
                             start=True, stop=True)
         zzT_sb = spool.tile([128, 2, ng], f32, tag="zzt")
         nc.vector.tensor_copy(zzT_sb[:], zzT_ps[:])

         z2_ps = ps_sm.tile([ng, H2], f32, tag="sm")
         for hh in range(2):
            nc.tensor.matmul(z2_ps[:], zzT_sb[:, hh, :], We2_sb[:, hh, :],
                             start=(hh == 0), stop=(hh == 1) and not with_bias)
         if with_bias:
            nc.tensor.matmul(z2_ps[:], onesrf[:, :ng], be2_row[:],
                             start=False, stop=True)
         z2l = spool.tile([ng, H2], f32, tag="z2l")
         nc.scalar.activation(z2l[:], z2_ps[:], AF.Prelu, alpha=0.2)
         z2T_ps = ps_sm.tile([H2, ng], f32, tag="sm")
         nc.tensor.matmul(z2T_ps[:], z2l[:], ident[:ng, :ng], is_transpose=True,
                         start=True, stop=True)
         z2T_sb = spool.tile([H2, ng], f32, tag="z2t")
         nc.vector.tensor_copy(z2T_sb[:], z2T_ps[:])

         y_ps = ps_sm.tile([ng, 1], f32, tag="sm")
         nc.tensor.matmul(y_ps[:], z2T_sb[:], We3_sb[:], start=True,
                         stop=not with_bias)
         if with_bias:
            nc.tensor.matmul(y_ps[:], onesrf[:, :ng], be3_row[:],
                             start=False, stop=True)
         y_sb = spool.tile([ng, 1], f32, tag="y")
         nc.vector.tensor_copy(y_sb[:], y_ps[:])
         nc.sync.dma_start(out_d[:], y_sb[:])

    nc.compile()
    _BUILD_CACHE[key] = nc
    return nc


_PARAM_KEYS = ("W1", "W2", "W3", "We1", "We2", "We3")
_BIAS_KEYS = ("b1", "b2", "b3", "be1", "be2", "be3")


def _fold_sd(W, a, b):
    """Wsd[k, 0:5] = sum_c W[k,(r,c)] a[r,c]; [:, 5:10] dst half. bsd likewise."""
    W = np.asarray(W, np.float64)
    a = np.asarray(a, np.float64)
    cin = W.shape[0]
    Wr = W.reshape(cin, R, C)
    asrc, adst = a[:, :C], a[:, C:]
    Wsrc = np.einsum("krc,rc->kr", Wr, asrc)
    Wdst = np.einsum("krc,rc->kr", Wr, adst)
    Wsd = np.concatenate([Wsrc, Wdst], axis=1).astype(np.float32)
    br = np.asarray(b, np.float64).reshape(R, C)
    bsd = np.concatenate(
        [np.einsum("rc,rc->r", br, asrc), np.einsum("rc,rc->r", br, adst)]
    ).reshape(1, NB).astype(np.float32)
    return np.ascontiguousarray(Wsd), np.ascontiguousarray(bsd)


def _transpose_bonds(yb):
    """[ng, i, j, r] i32 -> [ng, j', b=2r+jh, i] i32 (j = jh*128 + j')."""
    ng = yb.shape[0]
    bt = yb.transpose(0, 2, 3, 1)            # [ng, j, r, i]
    bt = bt.reshape(ng, 2, 128, R, N)        # [ng, jh, j', r, i]
    bt = bt.transpose(0, 2, 3, 1, 4)         # [ng, j', r, jh, i]
    return np.ascontiguousarray(bt.reshape(ng, 128, NB, N), np.int32)


def _shard_inputs(inputs, with_bias, n_cores, ng):
    wsd = {}
    for i in (1, 2, 3):
        wsd[f"Wsd{i}"], wsd[f"bsd{i}"] = _fold_sd(
            inputs[f"W{i}"], inputs[f"a{i}"], inputs[f"b{i}"]
        )
    per_core = []
    for c in range(n_cores):
        s = slice(c * ng, (c + 1) * ng)
        m = {
            "y_atoms": np.ascontiguousarray(inputs["y_atoms"][s], np.float32),
            "bonds_t": _transpose_bonds(np.asarray(inputs["y_bonds"][s], np.int32)),
            "x": np.ascontiguousarray(inputs["x"][s], np.float32),
        }
        for k in _PARAM_KEYS:
            m[k] = np.ascontiguousarray(inputs[k], np.float32)
        for i in (1, 2, 3):
            m[f"Wsd{i}"] = wsd[f"Wsd{i}"]
        if with_bias:
            for k in _BIAS_KEYS:
                m[k] = np.ascontiguousarray(np.asarray(inputs[k], np.float32).reshape(1, -1))
            for i in (1, 2, 3):
                m[f"bsd{i}"] = wsd[f"bsd{i}"]
        per_core.append(m)
    return per_core


def _needs_bias(inputs):
    return any(np.abs(np.asarray(inputs[k])).max() > 0 for k in _BIAS_KEYS)


def kernel(**inputs):
    from concourse.bass_utils import run_bass_kernel_spmd

    with_bias = _needs_bias(inputs)
    nc = build(NG, with_bias)
    in_maps = _shard_inputs(inputs, with_bias, NCORE, NG)
    res = run_bass_kernel_spmd(nc, in_maps, core_ids=list(range(NCORE)))
    out = np.concatenate([r["out"] for r in res.results], axis=0)
    return np.ascontiguousarray(out, np.float32)
